# revision 3
# baseline (speedup 1.0000x reference)
"""Trainium2 Bass kernel for nn_AppearanceComposability (sparse_attention).

Reference semantics, per (b, c) with 64x64 images, 3x3 unfold (pad 1):
  out_flat[m] = K_flat[m] * qv[m // 9],   qv[i] = Q_flat[9*i + 4]
where K_flat / Q_flat are the per-channel flattened unfold blocks
(kk*4096 + l, channel order (C, kh, kw)).

v3 implementation (bf16 end-to-end; rel err ~2.9e-3 vs gate 2e-2):
  - three compute engines, balanced: ACT stretches qs[l]=qv[(s+l)//9]
    for 9 chunks (DVE then does flat contiguous 2x-packed TTs on them),
    DVE does 3+ chunks in broadcast mode, GPSIMD does ~6 chunks in
    broadcast mode (measured ~1.74 ns/elem on HW)
  - broadcast chunks read the query operand STRAIGHT out of the padded
    query image via a (9,0)-strided broadcast AP -- no qv arrays at all;
    query-x-wrap zeros become 9-wide run memsets on the out tile
  - per-chunk out tiles with margins absorb the bcast group overhang
    (no cross-chunk ordering hazards)
  - loads: first halves of q0/k0 land in parallel on both HWDGE queues,
    the rest chained on sync; stores alternate across both queues, the
    scalar-queue issues deferred into the ACT stream between stretches
    so ACT never stalls on a not-yet-ready tile
  - data parallel over batch: 8 cores, core b handles batch b
"""
import os
import sys

import numpy as np


def _ensure_path():
    try:
        import concourse  # noqa: F401
    except ImportError:
        for p in ("/opt/trn_rl_repo", "/root/.axon_site/_ro/trn_rl_repo"):
            if os.path.isdir(p):
                sys.path.insert(0, p)
                return


_ensure_path()

import concourse.bacc as bacc  # noqa: E402
import concourse.tile as tile  # noqa: E402
from concourse import mybir  # noqa: E402
from concourse.bass_utils import run_bass_kernel_spmd  # noqa: E402
from concourse.tile import add_dep_helper  # noqa: E402


def _install_ntff_hook_shim():
    """Provide antenv.axon_hooks when the image's antenv lacks it."""
    try:
        import antenv.axon_hooks  # noqa: F401
        return
    except ImportError:
        pass
    try:
        import types

        import antenv
        holder = {"hook": None, "tried": False}

        def set_axon_ntff_profile_hook(h):
            holder["hook"] = h
            holder["tried"] = True

        def get_axon_ntff_profile_hook():
            if not holder["tried"]:
                holder["tried"] = True
                try:
                    from trn_agent_boot.trn_boot import _ntff_profile_via_ctypes
                    so = "/opt/axon/libaxon_pjrt.so"
                    if os.path.exists(so):
                        holder["hook"] = _ntff_profile_via_ctypes(so)
                except Exception:
                    holder["hook"] = None
            return holder["hook"]

        mod = types.ModuleType("antenv.axon_hooks")
        mod.set_axon_ntff_profile_hook = set_axon_ntff_profile_hook
        mod.get_axon_ntff_profile_hook = get_axon_ntff_profile_hook
        sys.modules["antenv.axon_hooks"] = mod
        antenv.axon_hooks = mod
    except Exception:
        pass


_install_ntff_hook_shim()

F32 = mybir.dt.float32
BF16 = mybir.dt.bfloat16

B = 8          # batch == number of cores
C = 256        # channels
H = W = 64
L = H * W      # 4096 pixels
K2 = 9         # 3x3 patch
M = L * K2     # 36864 per-channel output length
MARG = 80      # input image margin (>= 73 needed)
OM = 8         # output tile margin (>= 8 needed)
OTAIL = 584    # out tile tail pad so run-zero rearrange views stay in-bounds
QM = 8         # qs tile head margin
QTAIL = 580    # qs tile tail pad (same reason)
OFFS = [(kh - 1) * W + (kw - 1) for kh in range(3) for kw in range(3)]


def _ceil_div(a, b):
    return -(-a // b)


def _plan_qv_ops():
    """Per kk: (i_lo, i_hi, src_start, memsets) for qv[i] = Q_flat[9i+4].

    src position (relative to q image start at MARG) of qv[i] is
    src_start + 9*(i - i_lo).  memsets are (first, cnt, 64) runs in
    i-space where the query tap wraps an x-edge (must read as zero).
    """
    ops = []
    for kk in range(K2):
        s = L * kk
        i_lo = max(0, _ceil_div(s - 4, 9))
        i_hi = min(L, _ceil_div(s + L - 4, 9))
        src_start = 9 * i_lo + 4 - s + OFFS[kk]
        memsets = []
        kw = kk % 3
        if kw != 1:
            target = 0 if kw == 0 else 63
            i0 = (57 * (target - 4 + s)) % 64  # 57 = 9^-1 mod 64
            first = i_lo + ((i0 - i_lo) % 64)
            if first < i_hi:
                cnt = (i_hi - 1 - first) // 64 + 1
                memsets.append((first, cnt, 64))
        ops.append((i_lo, i_hi, src_start, memsets))
    return ops


def _plan_tt_ops():
    """Per kk: (g_lo, g_hi, ngroups, q0); TT covers l in [g_lo, g_hi)."""
    ops = []
    for kk in range(K2):
        s = L * kk
        g_lo = -(s % 9)
        g_hi = L + ((-(s + L)) % 9)
        ops.append((g_lo, g_hi, (g_hi - g_lo) // 9, (s + g_lo) // 9))
    return ops


QV_OPS = _plan_qv_ops()
TT_OPS = _plan_tt_ops()

# Per-(group, chunk) compute mode:
#   's'  = stretched: ACT builds qs, DVE flat 2x TT
#   'bD' = broadcast TT on DVE (1x)
#   'bG' = broadcast TT on GPSIMD (~1.74 ns/elem)
MODES = {
    0: {0: 'bG', 1: 's', 2: 'bG', 3: 's', 4: 's', 5: 's',
        6: 'bD', 7: 'bG', 8: 's'},
    1: {0: 'bG', 1: 's', 2: 'bG', 3: 's', 4: 'bD', 5: 's',
        6: 'bD', 7: 'bG', 8: 's'},
}

# Chunk processing order per engine.
ACT_ORDER = [(0, 1), (0, 3), (0, 4), (0, 5), (0, 8),
             (1, 1), (1, 3), (1, 5), (1, 8)]
GP_ORDER = [(0, 0), (0, 2), (0, 7), (1, 0), (1, 2), (1, 7)]
# DVE master schedule: flat TTs follow ACT stretches; broadcast chunks
# fill DVE's idle gaps; (1, 7)'s tail piece lands on DVE at the end.
DVE_STEPS = [(0, 1), (0, 6), (0, 3), (0, 4), (0, 5), (0, 8),
             (1, 1), (1, 3), (1, 4), (1, 5), (1, 6), (1, 7), (1, 8)]

# Store queue per (g, kk): 'y' = sync, 'c' = scalar (deferred issue),
# '2' = split halves across both queues (tail drain).
STORE_Q = {
    (0, 0): 'c', (0, 1): 'y', (0, 2): 'y', (0, 3): 'c', (0, 4): 'y',
    (0, 5): 'c', (0, 6): 'y', (0, 7): 'c', (0, 8): 'y',
    (1, 0): 'c', (1, 1): 'y', (1, 2): 'c', (1, 3): 'y', (1, 4): 'c',
    (1, 5): 'y', (1, 6): 'c', (1, 7): '2', (1, 8): '2',
}
# GP chunks whose stores get issued right after a given DVE step's store
# (matches estimated completion order so each queue's FIFO stays sane).
GP_STORE_AFTER = {
    (0, 1): [(0, 0)], (0, 6): [(0, 2)], (0, 5): [(0, 7)],
    (1, 1): [(1, 0)], (1, 4): [(1, 2)],
}

# Head splits (group 0 only): the first stretched / GP chunk is split so
# piece A only needs the first half-image loads. qv-group cut points.
GP_HEAD_CUT = 230    # (0,0): groups [0, cut) need only q0h1/k0h1
ACT_HEAD_CUT = 685   # (0,1): qv groups [455, cut) need only q0h1
# (1,7) partial: GP covers groups [i0, i0+GP_TAIL_CUT), DVE the rest.
GP_TAIL_CUT = 304


def build_graph():
    nc = bacc.Bacc(None, target_bir_lowering=False)
    key_ext = nc.declare_dram_parameter("key_map", [C, L], BF16,
                                        isOutput=False)
    query_ext = nc.declare_dram_parameter("query_map", [C, L], BF16,
                                          isOutput=False)
    out_ext = nc.declare_dram_parameter("out", [C, M], BF16, isOutput=True)

    ngroups = C // 128
    with tile.TileContext(nc) as tc:
        with (
            tc.tile_pool(name="pads", bufs=1) as pads,
            tc.tile_pool(name="qsp", bufs=3) as qsp,
            tc.tile_pool(name="outs", bufs=14) as outs,
        ):
            key_pads, q_pads = [], []
            for g in range(ngroups):
                q_pad = pads.tile([128, MARG + L + MARG], BF16,
                                  name=f"q_pad{g}", tag=f"q_pad{g}")
                nc.vector.memset(q_pad[:, 0:MARG], 0.0)
                nc.vector.memset(q_pad[:, MARG + L:MARG + L + MARG], 0.0)
                key_pad = pads.tile([128, MARG + L + MARG], BF16,
                                    name=f"key_pad{g}", tag=f"key_pad{g}")
                nc.vector.memset(key_pad[:, 0:MARG], 0.0)
                nc.vector.memset(key_pad[:, MARG + L:MARG + L + MARG], 0.0)
                key_pads.append(key_pad)
                q_pads.append(q_pad)

            # Loads. First halves of q0 (sync queue) and k0 (scalar
            # queue) land in parallel; the remaining six chained on the
            # sync queue so each completes before the next starts.
            hL = L // 2
            nc.scalar.dma_start(key_pads[0][:, MARG:MARG + hL],
                                key_ext[0:128, 0:hL])
            seq = [(0, "q", 0), (0, "q", 1), (0, "k", 1),
                   (1, "q", 0), (1, "q", 1), (1, "k", 0), (1, "k", 1)]
            prev_q = None
            for (g, t, h) in seq:
                pad = q_pads[g] if t == "q" else key_pads[g]
                ext = query_ext if t == "q" else key_ext
                qd = nc.sync.dma_start(
                    pad[:, MARG + h * hL:MARG + (h + 1) * hL],
                    ext[g * 128:(g + 1) * 128, h * hL:(h + 1) * hL])
                if prev_q is not None:
                    add_dep_helper(qd.ins, prev_q.ins, sync=True,
                                   reason="chain loads on sync queue")
                prev_q = qd

            # ---------- emission helpers ----------
            state = {"act": None, "gp": None, "dve": None}

            def chain(engine_key, op):
                prev = state[engine_key]
                if prev is not None:
                    add_dep_helper(op.ins, prev.ins, sync=False,
                                   reason=f"{engine_key} stream order")
                state[engine_key] = op
                return op

            def emit_stretch(g, qs, s, a, b):
                """ACT copies building qs[QM + 9i - s .. +9] = qv[i] for
                qv-groups i in [a, b), reading straight out of q_pad."""
                for kk2 in range(K2):
                    i_lo, i_hi, src_start, _ = QV_OPS[kk2]
                    c, dd = max(i_lo, a), min(i_hi, b)
                    if c >= dd:
                        continue
                    dst = qs[:, QM + 9 * c - s:
                             QM + 9 * dd - s].rearrange(
                        "p (n k) -> p n k", k=9)
                    sa = MARG + src_start + 9 * (c - i_lo)
                    src = q_pads[g][:, sa:sa + 9 * (dd - c):9].unsqueeze(
                        2).broadcast_to([128, dd - c, 9])
                    chain("act", nc.scalar.copy(dst, src))

            def emit_qs_runzeros(g, qs, s, a, b):
                """Zero 9-wide qs runs where the query tap wrapped an
                x-edge (DVE memsets; must precede the flat TT)."""
                for kk2 in range(K2):
                    i_lo, i_hi, _, msets = QV_OPS[kk2]
                    c, dd = max(i_lo, a), min(i_hi, b)
                    if c >= dd:
                        continue
                    for (first, cnt, step) in msets:
                        j0 = max(0, _ceil_div(c - first, step))
                        j1 = (dd - 1 - first) // step
                        if j0 > j1:
                            continue
                        iz = first + j0 * step
                        cnt2 = j1 - j0 + 1
                        A = QM + 9 * iz - s
                        view = qs[:, A:A + 576 * cnt2].rearrange(
                            "p (n k) -> p n k", k=576)[:, :, 0:9]
                        chain("dve", nc.vector.memset(view, 0.0))

            def emit_bcast(eng_name, g, kk, ot, a, b):
                """Broadcast TT for qv-groups [a, b) of chunk (g, kk),
                query operand read directly from q_pad; followed by the
                query-x-wrap run zeros on the out tile. eng_name in
                ('gp', 'dve')."""
                eng = nc.gpsimd if eng_name == "gp" else nc.vector
                s = kk * L
                key_pad = key_pads[g]
                for kk2 in range(K2):
                    i_lo, i_hi, src_start, _ = QV_OPS[kk2]
                    c, dd = max(i_lo, a), min(i_hi, b)
                    if c >= dd:
                        continue
                    n = dd - c
                    dst = ot[:, OM + 9 * c - s:
                             OM + 9 * dd - s].rearrange(
                        "p (n k) -> p n k", k=9)
                    src_k = key_pad[:, MARG + 9 * c - s + OFFS[kk]:
                                    MARG + 9 * dd - s + OFFS[kk]].rearrange(
                        "p (n k) -> p n k", k=9)
                    sa = MARG + src_start + 9 * (c - i_lo)
                    src_q = q_pads[g][:, sa:sa + 9 * n:9].unsqueeze(
                        2).broadcast_to([128, n, 9])
                    chain(eng_name, eng.tensor_mul(dst, src_k, src_q))
                # query-x-wrap zeros: 9-wide runs on the out tile
                for kk2 in range(K2):
                    i_lo, i_hi, _, msets = QV_OPS[kk2]
                    c, dd = max(i_lo, a), min(i_hi, b)
                    if c >= dd:
                        continue
                    for (first, cnt, step) in msets:
                        j0 = max(0, _ceil_div(c - first, step))
                        j1 = (dd - 1 - first) // step
                        if j0 > j1:
                            continue
                        iz = first + j0 * step
                        cnt2 = j1 - j0 + 1
                        A = OM + 9 * iz - s
                        view = ot[:, A:A + 576 * cnt2].rearrange(
                            "p (n k) -> p n k", k=576)[:, :, 0:9]
                        chain(eng_name, eng.memset(view, 0.0))

            def emit_colmset(eng_name, kk, ot):
                """Key-x-wrap zeros: stride-64 columns on the out tile."""
                kw = kk % 3
                if kw == 1:
                    return
                eng = nc.gpsimd if eng_name == "gp" else nc.vector
                c0 = 0 if kw == 0 else 63
                chain(eng_name,
                      eng.memset(ot[:, OM + c0:OM + L:64], 0.0))

            # ---------- tiles / stores ----------
            ots = {}

            def get_ot(g, kk):
                if (g, kk) not in ots:
                    ots[(g, kk)] = outs.tile(
                        [128, OM + L + OM + OTAIL], BF16,
                        name=f"ot{g}_{kk}", tag="ot")
                return ots[(g, kk)]

            pending_scalar = []

            def flush_scalar():
                while pending_scalar:
                    nc.scalar.dma_start(*pending_scalar.pop(0))

            def emit_store(g, kk):
                ot = ots[(g, kk)]
                rows = slice(g * 128, (g + 1) * 128)
                q = STORE_Q[(g, kk)]
                c0 = kk * L
                if q == 'y':
                    nc.sync.dma_start(out_ext[rows, c0:c0 + L],
                                      ot[:, OM:OM + L])
                elif q == 'c':
                    pending_scalar.append(
                        (out_ext[rows, c0:c0 + L], ot[:, OM:OM + L]))
                else:  # '2': split halves across both queues
                    nc.sync.dma_start(out_ext[rows, c0:c0 + hL],
                                      ot[:, OM:OM + hL])
                    nc.scalar.dma_start(out_ext[rows, c0 + hL:c0 + L],
                                        ot[:, OM + hL:OM + L])

            # ---------- GP stream (its own independent chain) ----------
            for (g, kk) in GP_ORDER:
                ot = get_ot(g, kk)
                glo, ghi, ng, q0 = TT_OPS[kk]
                s = kk * L
                i0 = (s + glo) // 9
                i1 = i0 + ng
                if (g, kk) == (0, 0):
                    emit_bcast("gp", g, kk, ot, i0, GP_HEAD_CUT)
                    emit_bcast("gp", g, kk, ot, GP_HEAD_CUT, i1)
                elif (g, kk) == (1, 7):
                    emit_bcast("gp", g, kk, ot, i0, i0 + GP_TAIL_CUT)
                else:
                    emit_bcast("gp", g, kk, ot, i0, i1)
                emit_colmset("gp", kk, ot)

            # ---------- ACT + DVE master schedule ----------
            stretch_idx = {ch: i for i, ch in enumerate(ACT_ORDER)}
            act_state = {"next": 0}
            qs_tiles = {}

            def ensure_stretch_upto(j):
                while act_state["next"] <= j:
                    (g, kk) = ACT_ORDER[act_state["next"]]
                    s = kk * L
                    qs = qsp.tile([128, QM + L + QM + QTAIL], BF16,
                                  name=f"qs{g}_{kk}", tag="qs")
                    qs_tiles[(g, kk)] = qs
                    i0 = s // 9
                    i1 = _ceil_div(s + L, 9)
                    if (g, kk) == (0, 1):
                        emit_stretch(g, qs, s, i0, ACT_HEAD_CUT)
                        emit_stretch(g, qs, s, ACT_HEAD_CUT, i1)
                    else:
                        emit_stretch(g, qs, s, i0, i1)
                    act_state["next"] += 1
                    flush_scalar()

            for step in DVE_STEPS:
                g, kk = step
                ot = get_ot(g, kk)
                if step in stretch_idx:
                    ensure_stretch_upto(min(stretch_idx[step] + 1,
                                            len(ACT_ORDER) - 1))
                    flush_scalar()
                    s = kk * L
                    qs = qs_tiles[step]
                    i0 = s // 9
                    i1 = _ceil_div(s + L, 9)
                    key_pad = key_pads[g]
                    if step == (0, 1):
                        cut = 9 * ACT_HEAD_CUT - s - 4
                        cut -= cut % 2
                        emit_qs_runzeros(g, qs, s, i0, ACT_HEAD_CUT)
                        chain("dve", nc.vector.tensor_mul(
                            ot[:, OM:OM + cut],
                            key_pad[:, MARG + OFFS[kk]:
                                    MARG + OFFS[kk] + cut],
                            qs[:, QM:QM + cut]))
                        emit_qs_runzeros(g, qs, s, ACT_HEAD_CUT, i1)
                        chain("dve", nc.vector.tensor_mul(
                            ot[:, OM + cut:OM + L],
                            key_pad[:, MARG + OFFS[kk] + cut:
                                    MARG + OFFS[kk] + L],
                            qs[:, QM + cut:QM + L]))
                    else:
                        emit_qs_runzeros(g, qs, s, i0, i1)
                        chain("dve", nc.vector.tensor_mul(
                            ot[:, OM:OM + L],
                            key_pad[:, MARG + OFFS[kk]:
                                    MARG + OFFS[kk] + L],
                            qs[:, QM:QM + L]))
                    emit_colmset("dve", kk, ot)
                elif step == (1, 7):
                    flush_scalar()
                    glo, ghi, ng, q0 = TT_OPS[kk]
                    i0 = (kk * L + glo) // 9
                    emit_bcast("dve", g, kk, ot,
                               i0 + GP_TAIL_CUT, i0 + ng)
                else:  # bD
                    flush_scalar()
                    glo, ghi, ng, q0 = TT_OPS[kk]
                    i0 = (kk * L + glo) // 9
                    emit_bcast("dve", g, kk, ot, i0, i0 + ng)
                    emit_colmset("dve", kk, ot)
                emit_store(g, kk)
                for gpch in GP_STORE_AFTER.get(step, []):
                    emit_store(*gpch)
            flush_scalar()
    nc.compile()
    return nc


_GRAPH_CACHE = {}


def _get_graph():
    if "nc" not in _GRAPH_CACHE:
        _GRAPH_CACHE["nc"] = build_graph()
    return _GRAPH_CACHE["nc"]


def kernel(key_map: np.ndarray, query_map: np.ndarray,
           _trace: bool = False, _tmpdir: str | None = None):
    import ml_dtypes
    bf16 = ml_dtypes.bfloat16
    key_map = np.ascontiguousarray(key_map, dtype=np.float32).astype(bf16)
    query_map = np.ascontiguousarray(query_map, dtype=np.float32).astype(bf16)
    assert key_map.shape == (B, C, H, W), key_map.shape

    nc = _get_graph()
    in_maps = [
        {"key_map": key_map[b].reshape(C, L),
         "query_map": query_map[b].reshape(C, L)}
        for b in range(B)
    ]
    res = run_bass_kernel_spmd(
        nc, in_maps, core_ids=list(range(B)),
        trace=_trace, tmpdir=_tmpdir,
    )
    out = np.stack([res.results[b]["out"] for b in range(B)])
    _GRAPH_CACHE["last_exec_time_ns"] = res.exec_time_ns
    _GRAPH_CACHE["last_results"] = res
    return out.astype(np.float32).reshape(B, C, L, K2)
